# revision 9
# baseline (speedup 1.0000x reference)
"""GRU encoder (ragged sequences) on 8 Trainium2 NeuronCores.

Strategy: pure data parallel over the sentence axis. On the host we sort
sentences by length (descending), pad each length-run to a multiple of 16
and deal the sorted list round-robin across the 8 cores so every core sees
the identical column-length profile (one SPMD program). On device, each
512-sentence tile runs the GRU scan with a shrinking active width: at step
t only columns whose length > t are updated, so finished sentences simply
stop being touched — no per-step masking, and total column-steps equal
sum(lengths). Matmuls run in bf16 (fp32 PSUM accumulate); gate math uses
ACT (sigmoid/tanh/prelu with fused per-partition bias), DVE (fused
scalar_tensor_tensor ops), and GPSIMD for the off-critical-path blend
terms. All 9 tiles are interleaved so the Tile scheduler can hide the
per-step serial dependency chain across independent tiles."""

import sys

for _p in ("/opt/trn_rl_repo",):
    if _p not in sys.path:
        sys.path.insert(0, _p)

import numpy as np

NC_N = 8
W = 512
SLOPE = 1e-4

_cache = {}

# kernel-structure knobs (tuned via cost-model sim)
CFG = {
    "group": 9,        # tiles interleaved per scheduling group
    "mm_bf16": True,   # matmul operands in bf16 instead of float32r
    "fc_inline": True,  # emit each tile's FC epilogue right after its last step
    "work_bufs": 5,
    "xin_bufs": 11,
    "split_cols": 0,     # leading (longest) columns get half-width tiles
    "split_w": 256,      # ...of this width, to parallelize the scan tail
    "xt_block": 8,       # steps of x loaded per DMA (fewer DMA-wait edges)
    "z_pack": 256,       # b <= this: z lands at rzp[b:2b] -> ONE sigmoid [0:2b]
    "act_split_max": 340,  # z_pack < b <= this: two sigmoids; above: fused [0:W+b]
    "t3_pool": True,     # z*h product on gpsimd (off critical path)
    "relu_pool_mod": 2,  # every k-th step's relu runs on gpsimd (0 = never)
    "deal_top": 4,       # equalize length profile of the first K tiles so
                         # multiple scan chains stay alive in the endgame
}


# ---------------------------------------------------------------------------
# Workaround for this container's walrus codegen: setupSyncWait rejects
# instructions carrying more than one sync-wait. After Tile's semaphore
# assignment, hoist excess waits onto same-engine NoOps inserted just
# before the over-subscribed instruction.
_MAXW = 1
_MAXW_COMPUTE = 1
_patched = [False]


def _apply_tile_patch():
    if _patched[0]:
        return
    import concourse.tile as tile
    import concourse.mybir as mybir

    counter = [0]

    def _split(insts):
        out = []
        for inst in insts:
            si = inst.sync_info
            maxw = 1 if isinstance(inst, mybir.InstDrain) else _MAXW_COMPUTE
            if si is not None and si.on_wait and len(si.on_wait) > maxw:
                waits = list(si.on_wait)
                keep, excess = waits[-maxw:], waits[:-maxw]
                while excess:
                    chunk, excess = excess[:_MAXW], excess[_MAXW:]
                    counter[0] += 1
                    out.append(
                        mybir.InstNoOp(
                            name=f"bass-waitsplit-{counter[0]}",
                            engine=inst.engine,
                            sync_info=mybir.SyncInfo(on_wait=chunk, on_update=[]),
                        )
                    )
                si.on_wait = keep
            out.append(inst)
        return out

    orig_lower = tile.TileContext._lower_ordered_insts

    def patched_lower(self, ordered):
        for name in list(ordered.keys()):
            ordered[name] = _split(ordered[name])
        return orig_lower(self, ordered)

    def patched_dab(self, tick_clock, wait_clock):
        drain_inst = self.nc.sync.drain()
        wait_clock.add_sem_waits(
            drain_inst.ins, tile.ScopedClock({None: tick_clock.global_clock})
        )
        si = drain_inst.ins.sync_info
        if si is not None and si.on_wait and len(si.on_wait) > _MAXW:
            waits = list(si.on_wait)
            si.on_wait = waits[:_MAXW]
            rest = waits[_MAXW:]
            while rest:
                chunk, rest = rest[:_MAXW], rest[_MAXW:]
                extra = self.nc.sync.drain()
                extra.ins.sync_info = mybir.SyncInfo(on_wait=chunk, on_update=[])
        self.nc.all_engine_barrier()
        assert self.sems is not None
        popped = self.nc._tile_sem_poison_stack.pop()
        assert popped is self._sem_poison
        self.nc.clear_and_free_semaphores(list(self.sems.allocated().values()))
        self.nc.all_engine_barrier()

    tile.TileContext._lower_ordered_insts = patched_lower
    tile.TileContext._drain_and_barrier = patched_dab
    _patched[0] = True


# ---------------------------------------------------------------------------
def _prepare(lengths, T):
    """Sorted-desc global order with per-length-run padding to multiples of 8.

    Returns (g, M, tiles) where g[-1-padded global order, -1 = dummy],
    M = per-core column count, tiles = list of (c0, Wj, Lmax, widths)."""
    S = lengths.shape[0]
    # Pad each length-run to a multiple of 2*NC_N so that (a) every core gets
    # the identical column-length profile and (b) per-step active widths are
    # even (f32r matmul ISA requires even innermost element counts).
    RUN = 2 * NC_N
    order = np.argsort(-lengths, kind="stable")
    sorted_lens = lengths[order]
    entries = []
    ent_lens = []
    maxlen = int(lengths.max(initial=0))
    for L in range(maxlen, 0, -1):
        idx = order[sorted_lens == L]
        if len(idx) == 0:
            continue
        entries.extend(int(i) for i in idx)
        ent_lens.extend([L] * len(idx))
        pad = (-len(idx)) % RUN
        entries.extend([-1] * pad)
        ent_lens.extend([L] * pad)
    idx0 = order[sorted_lens == 0]
    entries.extend(int(i) for i in idx0)
    ent_lens.extend([0] * len(idx0))
    pad = (-len(entries)) % RUN
    entries.extend([-1] * pad)
    ent_lens.extend([0] * pad)
    g = np.asarray(entries, dtype=np.int64)
    glen = np.asarray(ent_lens, dtype=np.int64)
    M = len(g) // NC_N

    # Deal the longest columns round-robin (in pairs, keeping widths even)
    # across the first K tiles so all K run Lmax steps and the scan tail has
    # K parallel chains instead of one.
    K = CFG.get("deal_top", 0)
    if K > 1 and M >= K * W:
        n = K * W
        perm = np.empty(n, dtype=np.int64)
        for p in range(n):
            t_id, s = p // W, p % W
            q = (s // 2) * K + t_id
            perm[p] = 2 * q + (p % 2)
        gv = g.reshape(M, NC_N)
        lv = glen.reshape(M, NC_N)
        gv[:n] = gv[:n][perm]
        lv[:n] = lv[:n][perm]
        g = gv.reshape(-1)
        glen = lv.reshape(-1)

    len_col = glen[::NC_N]  # identical across cores by construction

    tiles = []
    c0 = 0
    while c0 < M:
        step = CFG["split_w"] if c0 < CFG["split_cols"] else W
        Wj = min(step, M - c0)
        seg = len_col[c0 : c0 + Wj]
        Lmax = int(seg.max(initial=0))
        widths = [int(np.sum(seg > t)) for t in range(Lmax)]
        tiles.append((c0, Wj, Lmax, tuple(widths)))
        c0 += Wj
    return g, M, tiles


def _build(M, T, tiles, fc_b_val, reps=1):
    import concourse.bass as bass
    import concourse.tile as tile
    import concourse.mybir as mybir

    _apply_tile_patch()
    F32 = mybir.dt.float32
    F32R = mybir.dt.bfloat16 if CFG["mm_bf16"] else mybir.dt.float32r
    BLD = mybir.dt.bfloat16
    Act = mybir.ActivationFunctionType
    Alu = mybir.AluOpType

    nc = bass.Bass("TRN2", target_bir_lowering=False, debug=False, num_devices=NC_N)
    xr_d = nc.declare_dram_parameter("x_r", [3, T, M], F32R, isOutput=False)
    whh_d = nc.declare_dram_parameter("whh_t", [128, 384], F32R, isOutput=False)
    wihb_d = nc.declare_dram_parameter("wihb", [3, 256], F32R, isOutput=False)
    wihn_d = nc.declare_dram_parameter("wih_n", [2, 128], F32R, isOutput=False)
    ident_d = nc.declare_dram_parameter("ident", [128, 128], F32R, isOutput=False)
    fcw_d = nc.declare_dram_parameter("fc_wT", [128, 128], F32R, isOutput=False)
    bias_d = nc.declare_dram_parameter("biases", [128, 3], F32, isOutput=False)
    out_d = nc.declare_dram_parameter("out", [1, M], F32, isOutput=True)

    with tile.TileContext(nc) as tc:
        with (
            tc.tile_pool(name="const", bufs=1) as cp,
            tc.tile_pool(name="hpool", bufs=1) as hp,
            tc.tile_pool(name="work", bufs=CFG["work_bufs"]) as sp,
            tc.tile_pool(name="xin", bufs=CFG["xin_bufs"]) as xp,
        ):
            whh = cp.tile([128, 384], F32R)
            nc.sync.dma_start(whh[:], whh_d[:])
            wihb = cp.tile([3, 256], F32R)
            nc.sync.dma_start(wihb[:], wihb_d[:])
            wihn = cp.tile([2, 128], F32R)
            nc.sync.dma_start(wihn[:], wihn_d[:])
            ident = cp.tile([128, 128], F32R)
            nc.sync.dma_start(ident[:], ident_d[:])
            fcw = cp.tile([128, 128], F32R)
            nc.sync.dma_start(fcw[:], fcw_d[:])
            bias = cp.tile([128, 3], F32)
            nc.sync.dma_start(bias[:], bias_d[:])
            b_in = bias[:, 0:1]
            b_hn = bias[:, 1:2]
            b_fc = bias[:, 2:3]

            hts = []
            for j, (c0, Wj, Lmax, widths) in enumerate(tiles):
                ht = hp.tile([128, Wj], F32R, tag=f"h{j}")
                hts.append(ht)

            pp_box = {}
            xt_box = {}

            def emit_step(j, t, b):
                pp = pp_box["pp"]
                c0, Wj, Lmax, widths = tiles[j]
                h = hts[j]
                first = t == 0
                XB = CFG["xt_block"]
                t0 = (t // XB) * XB
                if xt_box.get(j, (None, None))[0] != t0:
                    nsteps = min(XB, Lmax - t0)
                    b0 = widths[t0]
                    xtb = xp.tile([3, XB * W], F32R, tag="xt")
                    dst = xtb[:].rearrange("p (s w) -> p s w", s=XB)
                    nc.sync.dma_start(
                        dst[:, 0:nsteps, 0:b0],
                        xr_d[:, t0 : t0 + nsteps, c0 : c0 + b0],
                    )
                    xt_box[j] = (t0, xtb)
                xtb = xt_box[j][1]
                off = (t - t0) * W
                xt = xtb[:, off : off + W]
                # r end-aligned in bank 0, z start-aligned in bank 1: the
                # r|z block [W-b : W+b] is contiguous for every b, so ONE
                # sigmoid instruction covers exactly 2b columns.
                roff = W - b
                rzp = pp.tile([128, 2 * W], F32, tag="rzp")
                if not first:
                    hnp = pp.tile([128, W], F32, tag="hnp")
                    nc.tensor.matmul(
                        rzp[:, roff:W], whh[:, 0:128], h[:, 0:b],
                        start=True, stop=False,
                    )
                    nc.tensor.matmul(
                        rzp[:, W : W + b], whh[:, 128:256], h[:, 0:b],
                        start=True, stop=False,
                    )
                nc.tensor.matmul(
                    rzp[:, roff:W], wihb[:, 0:128], xt[:, 0:b],
                    start=first, stop=True,
                )
                nc.tensor.matmul(
                    rzp[:, W : W + b], wihb[:, 128:256], xt[:, 0:b],
                    start=first, stop=True,
                )
                if not first:
                    nc.tensor.matmul(
                        hnp[:, 0:b], whh[:, 256:384], h[:, 0:b],
                        start=True, stop=True,
                    )
                rz = sp.tile([128, 2 * W], BLD, tag="rzs")
                nc.scalar.activation(
                    rz[:, roff : W + b], rzp[:, roff : W + b], Act.Sigmoid
                )
                rv = rz[:, roff:W]
                zv = rz[:, W : W + b]
                t1 = sp.tile([128, W], F32R, tag="t1")
                if first:
                    # h == 0: (hn + b_hn) * r == b_hn * r
                    nc.vector.tensor_scalar(
                        t1[:, 0:b], rv, b_hn, None, op0=Alu.mult
                    )
                else:
                    nc.vector.scalar_tensor_tensor(
                        t1[:, 0:b], hnp[:, 0:b], b_hn, rv,
                        op0=Alu.add, op1=Alu.mult,
                    )
                inp = pp.tile([128, W], F32, tag="inp")
                nc.tensor.matmul(
                    inp[:, 0:b], wihn[:, 0:128], xt[0:2, 0:b], start=True, stop=False
                )
                nc.tensor.matmul(
                    inp[:, 0:b], ident[:, 0:128], t1[:, 0:b], start=False, stop=True
                )
                # off-critical-path: zc = 1-z (DVE 4x), t3 = z*h (gpsimd)
                zc = sp.tile([128, W], BLD, tag="zc")
                nc.vector.tensor_scalar(
                    zc[:, 0:b], zv, -1.0, 1.0, op0=Alu.mult, op1=Alu.add
                )
                if not first:
                    t3 = sp.tile([128, W], BLD, tag="t3")
                    t3e = nc.gpsimd if CFG["t3_pool"] else nc.vector
                    t3e.tensor_mul(t3[:, 0:b], zv, h[:, 0:b])
                n = sp.tile([128, W], BLD, tag="n")
                nc.scalar.activation(n[:, 0:b], inp[:, 0:b], Act.Tanh, bias=b_in)
                t2 = sp.tile([128, W], BLD, tag="t2")
                nc.vector.tensor_mul(t2[:, 0:b], zc[:, 0:b], n[:, 0:b])
                # leaky(y) with slope 1e-4 ~= relu(y): max abs output error
                # ~1e-5, far inside tolerance; relu is a single cheap
                # tensor_scalar instead of ACT Prelu / DVE STT.
                kpool = CFG["relu_pool_mod"]
                relu_eng = (
                    nc.gpsimd if (kpool and t % kpool == 0) else nc.vector
                )
                if first:
                    relu_eng.tensor_scalar(
                        h[:, 0:b], t2[:, 0:b], 0.0, None, op0=Alu.max
                    )
                else:
                    y = sp.tile([128, W], BLD, tag="y")
                    nc.vector.tensor_add(y[:, 0:b], t2[:, 0:b], t3[:, 0:b])
                    relu_eng.tensor_scalar(
                        h[:, 0:b], y[:, 0:b], 0.0, None, op0=Alu.max
                    )

            def emit_fc(j, fc_pool):
                c0, Wj, Lmax, widths = tiles[j]
                if Lmax == 0:
                    return
                fcp = fc_pool.tile([128, W], F32,
                                   tag="hnp" if CFG["fc_inline"] else "fcp")
                nc.tensor.matmul(
                    fcp[:, 0:Wj], fcw[:, 0:128], hts[j][:, 0:Wj],
                    start=True, stop=True,
                )
                le = sp.tile([1, W], F32, tag="le")
                nc.scalar.activation(
                    le[0:1, 0:Wj], fcp[0:1, 0:Wj], Act.Prelu,
                    bias=b_fc[0:1, 0:1], alpha=SLOPE,
                )
                e = sp.tile([1, W], F32, tag="e")
                nc.scalar.activation(e[0:1, 0:Wj], le[0:1, 0:Wj], Act.Sigmoid)
                nc.sync.dma_start(out_d[0:1, c0 : c0 + Wj], e[0:1, 0:Wj])

            n_tiles = len(tiles)
            GRP = CFG["group"]
            for _rep in range(reps):
                pp_cm = tc.tile_pool(name=f"psum_scan{_rep}", bufs=2,
                                     space="PSUM")
                pp_box["pp"] = pp_cm.__enter__()
                for g0 in range(0, n_tiles, GRP):
                    group = list(range(g0, min(g0 + GRP, n_tiles)))
                    tmax = max(tiles[j][2] for j in group)
                    for t in range(tmax):
                        for j in group:
                            if t < tiles[j][2]:
                                b = tiles[j][3][t]
                                if b > 0:
                                    emit_step(j, t, b)
                                if CFG["fc_inline"] and t == tiles[j][2] - 1:
                                    emit_fc(j, pp_box["pp"])
                    if CFG["fc_inline"]:
                        for j in group:
                            if tiles[j][2] == 0:
                                pass  # zero-length tiles have no fc
                pp_cm.__exit__(None, None, None)

                if not CFG["fc_inline"]:
                    # FC + leaky + sigmoid epilogue at the end
                    pf_cm = tc.tile_pool(name=f"psum_fc{_rep}", bufs=2,
                                         space="PSUM")
                    pf = pf_cm.__enter__()
                    for j in range(n_tiles):
                        emit_fc(j, pf)
                    pf_cm.__exit__(None, None, None)

    return nc


def _run(x, lengths, w_ih, w_hh, b_ih, b_hh, fc_w, fc_b, trace=False):
    from concourse.bass_utils import run_bass_kernel_spmd

    S, T, _ = x.shape
    H = w_hh.shape[1]
    g, M, tiles = _prepare(lengths, T)

    key = (S, T, M, tuple(tiles), float(fc_b[0]))
    if key not in _cache:
        _cache[key] = _build(M, T, tiles, float(fc_b[0]))
    nc = _cache[key]

    # shared weight arrays
    whh_t = np.ascontiguousarray(
        np.concatenate(
            [w_hh[0:H].T, w_hh[H : 2 * H].T, w_hh[2 * H : 3 * H].T], axis=1
        )
    ).astype(np.float32)  # [128, 384] columns r|z|n
    wihb = np.concatenate(
        [
            np.stack([w_ih[0:H, 0], w_ih[0:H, 1], b_ih[0:H] + b_hh[0:H]]),
            np.stack(
                [w_ih[H : 2 * H, 0], w_ih[H : 2 * H, 1],
                 b_ih[H : 2 * H] + b_hh[H : 2 * H]]
            ),
        ],
        axis=1,
    ).astype(np.float32)  # [3, 256]
    wih_n = np.ascontiguousarray(w_ih[2 * H : 3 * H].T).astype(np.float32)  # [2,128]
    ident = np.eye(128, dtype=np.float32)
    fc_wT = np.zeros((128, 128), dtype=np.float32)
    fc_wT[:, 0] = fc_w[0]  # only output row 0 of the FC matmul is read
    biases = np.stack(
        [b_ih[2 * H : 3 * H], b_hh[2 * H : 3 * H],
         np.full(H, fc_b[0], dtype=np.float32)], axis=1
    ).astype(np.float32)  # [128, 3] col0 b_in col1 b_hn col2 fc_b

    if CFG["mm_bf16"]:
        import ml_dtypes

        bf = ml_dtypes.bfloat16
        whh_t = whh_t.astype(bf)
        wihb = wihb.astype(bf)
        wih_n = wih_n.astype(bf)
        ident = ident.astype(bf)
        fc_wT = fc_wT.astype(bf)

    in_maps = []
    core_g = []
    for c in range(NC_N):
        idx = g[c::NC_N]
        core_g.append(idx)
        real = idx >= 0
        xg = np.zeros((M, T, 2), np.float32)
        xg[real] = x[idx[real]]
        xr = np.empty((3, T, M), np.float32)
        xr[0] = xg[:, :, 0].T
        xr[1] = xg[:, :, 1].T
        xr[2] = 1.0
        if CFG["mm_bf16"]:
            import ml_dtypes

            xr = xr.astype(ml_dtypes.bfloat16)
        in_maps.append(
            {
                "x_r": xr,
                "whh_t": whh_t,
                "wihb": wihb,
                "wih_n": wih_n,
                "ident": ident,
                "fc_wT": fc_wT,
                "biases": biases,
            }
        )

    global _last_in_maps
    _last_in_maps = in_maps
    res = run_bass_kernel_spmd(nc, in_maps, list(range(NC_N)), trace=trace)

    out_full = np.zeros(S, dtype=np.float32)
    for c in range(NC_N):
        idx = core_g[c]
        real = idx >= 0
        enc = res.results[c]["out"][0]
        out_full[idx[real]] = enc[real]
    out_full[lengths == 0] = np.float32(0.5)
    return out_full, res


def kernel(**inputs):
    x = np.asarray(inputs["x"], dtype=np.float32)
    lengths = np.asarray(inputs["lengths"], dtype=np.int32)
    w_ih = np.asarray(inputs["w_ih"], dtype=np.float32)
    w_hh = np.asarray(inputs["w_hh"], dtype=np.float32)
    b_ih = np.asarray(inputs["b_ih"], dtype=np.float32)
    b_hh = np.asarray(inputs["b_hh"], dtype=np.float32)
    fc_w = np.asarray(inputs["fc_w"], dtype=np.float32)
    fc_b = np.asarray(inputs["fc_b"], dtype=np.float32)
    out, _ = _run(x, lengths, w_ih, w_hh, b_ih, b_hh, fc_w, fc_b)
    return out



# revision 21
# speedup vs baseline: 2.1415x; 2.1415x over previous
"""GRU encoder (ragged sequences) on 8 Trainium2 NeuronCores.

Strategy: pure data parallel over the sentence axis. On the host we sort
sentences by length (descending), pad each length-run to a multiple of 16
and deal the sorted list round-robin across the 8 cores so every core sees
the identical column-length profile (one SPMD program). On device, each
512-sentence tile runs the GRU scan with a shrinking active width: at step
t only columns whose length > t are updated, so finished sentences simply
stop being touched — no per-step masking, and total column-steps equal
sum(lengths). The longest columns are dealt round-robin across the leading
K tiles (deal_top) so several scan chains stay alive through the tail.

Per step: r|z gate matmuls write PSUM end-aligned/start-aligned around a
bank boundary so ONE sigmoid covers the contiguous 2b block; the n-gate
pre-activation accumulates r*(U_n h + b_hn) through an identity matmul;
tanh applies b_in as fused bias. The blend h' = (1-z)*n + z*h runs in bf16
(DVE 2x/4x modes) with z*h on GPSIMD off the critical path, and the
per-step leaky-relu (slope 1e-4) is approximated by relu (max abs output
error ~1e-5), fused likewise into the FC epilogue as
sigmoid(leaky(v)) ~= max(sigmoid(v), 0.5). All tiles are interleaved so
the Tile scheduler hides each chain's serial latency."""

import sys

for _p in ("/opt/trn_rl_repo",):
    if _p not in sys.path:
        sys.path.insert(0, _p)

import numpy as np

NC_N = 8
W = 512
SLOPE = 1e-4

_cache = {}

# kernel-structure knobs (tuned via cost-model sim)
CFG = {
    "group": 9,        # tiles interleaved per scheduling group
    "mm_bf16": True,   # matmul operands in bf16 instead of float32r
    "fc_inline": True,  # emit each tile's FC epilogue right after its last step
    "work_bufs": 6,
    "xin_bufs": 11,
    "split_cols": 0,     # leading (longest) columns get half-width tiles
    "split_w": 256,      # ...of this width, to parallelize the scan tail
    "xt_block": 8,       # steps of x loaded per DMA (fewer DMA-wait edges)
    "z_pack": 256,       # b <= this: z lands at rzp[b:2b] -> ONE sigmoid [0:2b]
    "act_split_max": 340,  # z_pack < b <= this: two sigmoids; above: fused [0:W+b]
    "t3_pool": True,     # z*h product on gpsimd (off critical path)
    "relu_pool_mod": 0,
    "tail_split_t": 99,  # split the r|z sigmoid into two insts from this step on  # every k-th step's relu runs on gpsimd (0 = never)
    "deal_top": 2,       # equalize length profile of the first K tiles so
                         # multiple scan chains stay alive in the endgame
}


# ---------------------------------------------------------------------------
# Workaround for this container's walrus codegen: setupSyncWait rejects
# instructions carrying more than one sync-wait. After Tile's semaphore
# assignment, hoist excess waits onto same-engine NoOps inserted just
# before the over-subscribed instruction.
_MAXW = 1
_MAXW_COMPUTE = 1
_patched = [False]


def _apply_tile_patch():
    if _patched[0]:
        return
    import concourse.tile as tile
    import concourse.mybir as mybir

    counter = [0]

    def _split(insts):
        out = []
        for inst in insts:
            si = inst.sync_info
            maxw = 1 if isinstance(inst, mybir.InstDrain) else _MAXW_COMPUTE
            if si is not None and si.on_wait and len(si.on_wait) > maxw:
                waits = list(si.on_wait)
                keep, excess = waits[-maxw:], waits[:-maxw]
                while excess:
                    chunk, excess = excess[:_MAXW], excess[_MAXW:]
                    counter[0] += 1
                    out.append(
                        mybir.InstNoOp(
                            name=f"bass-waitsplit-{counter[0]}",
                            engine=inst.engine,
                            sync_info=mybir.SyncInfo(on_wait=chunk, on_update=[]),
                        )
                    )
                si.on_wait = keep
            out.append(inst)
        return out

    orig_lower = tile.TileContext._lower_ordered_insts

    def patched_lower(self, ordered):
        for name in list(ordered.keys()):
            ordered[name] = _split(ordered[name])
        return orig_lower(self, ordered)

    def patched_dab(self, tick_clock, wait_clock):
        drain_inst = self.nc.sync.drain()
        wait_clock.add_sem_waits(
            drain_inst.ins, tile.ScopedClock({None: tick_clock.global_clock})
        )
        si = drain_inst.ins.sync_info
        if si is not None and si.on_wait and len(si.on_wait) > _MAXW:
            waits = list(si.on_wait)
            si.on_wait = waits[:_MAXW]
            rest = waits[_MAXW:]
            while rest:
                chunk, rest = rest[:_MAXW], rest[_MAXW:]
                extra = self.nc.sync.drain()
                extra.ins.sync_info = mybir.SyncInfo(on_wait=chunk, on_update=[])
        self.nc.all_engine_barrier()
        assert self.sems is not None
        popped = self.nc._tile_sem_poison_stack.pop()
        assert popped is self._sem_poison
        self.nc.clear_and_free_semaphores(list(self.sems.allocated().values()))
        self.nc.all_engine_barrier()

    tile.TileContext._lower_ordered_insts = patched_lower
    tile.TileContext._drain_and_barrier = patched_dab
    _patched[0] = True


# ---------------------------------------------------------------------------
def _prepare(lengths, T):
    """Sorted-desc global order with per-length-run padding to multiples of 8.

    Returns (g, M, tiles) where g[-1-padded global order, -1 = dummy],
    M = per-core column count, tiles = list of (c0, Wj, Lmax, widths)."""
    S = lengths.shape[0]
    # Pad each length-run to a multiple of 2*NC_N so that (a) every core gets
    # the identical column-length profile and (b) per-step active widths are
    # even (f32r matmul ISA requires even innermost element counts).
    RUN = 2 * NC_N
    order = np.argsort(-lengths, kind="stable")
    sorted_lens = lengths[order]
    entries = []
    ent_lens = []
    maxlen = int(lengths.max(initial=0))
    for L in range(maxlen, 0, -1):
        idx = order[sorted_lens == L]
        if len(idx) == 0:
            continue
        entries.extend(int(i) for i in idx)
        ent_lens.extend([L] * len(idx))
        pad = (-len(idx)) % RUN
        entries.extend([-1] * pad)
        ent_lens.extend([L] * pad)
    idx0 = order[sorted_lens == 0]
    entries.extend(int(i) for i in idx0)
    ent_lens.extend([0] * len(idx0))
    pad = (-len(entries)) % RUN
    entries.extend([-1] * pad)
    ent_lens.extend([0] * pad)
    g = np.asarray(entries, dtype=np.int64)
    glen = np.asarray(ent_lens, dtype=np.int64)
    M = len(g) // NC_N

    # Deal the longest columns round-robin (in pairs, keeping widths even)
    # across the first K tiles so all K run Lmax steps and the scan tail has
    # K parallel chains instead of one.
    K = CFG.get("deal_top", 0)
    if K > 1 and M >= K * W:
        n = K * W
        perm = np.empty(n, dtype=np.int64)
        for p in range(n):
            t_id, s = p // W, p % W
            q = (s // 2) * K + t_id
            perm[p] = 2 * q + (p % 2)
        gv = g.reshape(M, NC_N)
        lv = glen.reshape(M, NC_N)
        gv[:n] = gv[:n][perm]
        lv[:n] = lv[:n][perm]
        g = gv.reshape(-1)
        glen = lv.reshape(-1)

    len_col = glen[::NC_N]  # identical across cores by construction

    tiles = []
    c0 = 0
    while c0 < M:
        step = CFG["split_w"] if c0 < CFG["split_cols"] else W
        Wj = min(step, M - c0)
        seg = len_col[c0 : c0 + Wj]
        Lmax = int(seg.max(initial=0))
        widths = [int(np.sum(seg > t)) for t in range(Lmax)]
        tiles.append((c0, Wj, Lmax, tuple(widths)))
        c0 += Wj
    return g, M, tiles


def _build(M, T, tiles, fc_b_val, reps=1):
    import concourse.bass as bass
    import concourse.tile as tile
    import concourse.mybir as mybir

    _apply_tile_patch()
    F32 = mybir.dt.float32
    F32R = mybir.dt.bfloat16 if CFG["mm_bf16"] else mybir.dt.float32r
    BLD = mybir.dt.bfloat16
    Act = mybir.ActivationFunctionType
    Alu = mybir.AluOpType

    nc = bass.Bass("TRN2", target_bir_lowering=False, debug=False, num_devices=NC_N)
    xr_d = nc.declare_dram_parameter("x_r", [3, T, M], F32R, isOutput=False)
    whh_d = nc.declare_dram_parameter("whh_t", [128, 384], F32R, isOutput=False)
    wihb_d = nc.declare_dram_parameter("wihb", [3, 256], F32R, isOutput=False)
    wihn_d = nc.declare_dram_parameter("wih_n", [2, 128], F32R, isOutput=False)
    ident_d = nc.declare_dram_parameter("ident", [128, 128], F32R, isOutput=False)
    fcw_d = nc.declare_dram_parameter("fc_wT", [128, 128], F32R, isOutput=False)
    bias_d = nc.declare_dram_parameter("biases", [128, 3], F32, isOutput=False)
    out_d = nc.declare_dram_parameter("out", [1, M], F32, isOutput=True)

    with tile.TileContext(nc) as tc:
        with (
            tc.tile_pool(name="const", bufs=1) as cp,
            tc.tile_pool(name="hpool", bufs=1) as hp,
            tc.tile_pool(name="work", bufs=CFG["work_bufs"]) as sp,
            tc.tile_pool(name="xin", bufs=CFG["xin_bufs"]) as xp,
        ):
            whh = cp.tile([128, 384], F32R)
            nc.sync.dma_start(whh[:], whh_d[:])
            wihb = cp.tile([3, 256], F32R)
            nc.sync.dma_start(wihb[:], wihb_d[:])
            wihn = cp.tile([2, 128], F32R)
            nc.sync.dma_start(wihn[:], wihn_d[:])
            ident = cp.tile([128, 128], F32R)
            nc.sync.dma_start(ident[:], ident_d[:])
            fcw = cp.tile([128, 128], F32R)
            nc.sync.dma_start(fcw[:], fcw_d[:])
            bias = cp.tile([128, 3], F32)
            nc.sync.dma_start(bias[:], bias_d[:])
            b_in = bias[:, 0:1]
            b_hn = bias[:, 1:2]
            b_fc = bias[:, 2:3]

            hts = []
            for j, (c0, Wj, Lmax, widths) in enumerate(tiles):
                ht = hp.tile([128, Wj], F32R, tag=f"h{j}")
                hts.append(ht)

            pp_box = {}
            xt_box = {}

            def emit_step(j, t, b):
                pp = pp_box["pp"]
                c0, Wj, Lmax, widths = tiles[j]
                h = hts[j]
                first = t == 0
                XB = CFG["xt_block"]
                t0 = (t // XB) * XB
                if xt_box.get(j, (None, None))[0] != t0:
                    nsteps = min(XB, Lmax - t0)
                    b0 = widths[t0]
                    xtb = xp.tile([3, XB * W], F32R, tag="xt")
                    dst = xtb[:].rearrange("p (s w) -> p s w", s=XB)
                    nc.sync.dma_start(
                        dst[:, 0:nsteps, 0:b0],
                        xr_d[:, t0 : t0 + nsteps, c0 : c0 + b0],
                    )
                    xt_box[j] = (t0, xtb)
                xtb = xt_box[j][1]
                off = (t - t0) * W
                xt = xtb[:, off : off + W]
                # r end-aligned in bank 0, z start-aligned in bank 1: the
                # r|z block [W-b : W+b] is contiguous for every b, so ONE
                # sigmoid instruction covers exactly 2b columns.
                roff = W - b
                rzp = pp.tile([128, 2 * W], F32, tag="rzp")
                if not first:
                    hnp = pp.tile([128, W], F32, tag="hnp")
                    nc.tensor.matmul(
                        rzp[:, roff:W], whh[:, 0:128], h[:, 0:b],
                        start=True, stop=False,
                    )
                    nc.tensor.matmul(
                        rzp[:, W : W + b], whh[:, 128:256], h[:, 0:b],
                        start=True, stop=False,
                    )
                nc.tensor.matmul(
                    rzp[:, roff:W], wihb[:, 0:128], xt[:, 0:b],
                    start=first, stop=True,
                )
                nc.tensor.matmul(
                    rzp[:, W : W + b], wihb[:, 128:256], xt[:, 0:b],
                    start=first, stop=True,
                )
                if not first:
                    nc.tensor.matmul(
                        hnp[:, 0:b], whh[:, 256:384], h[:, 0:b],
                        start=True, stop=True,
                    )
                rz = sp.tile([128, 2 * W], BLD, tag="rzs")
                if t >= CFG["tail_split_t"]:
                    # latency-bound tail: separate r so t1 starts before the
                    # z half of the sigmoid finishes
                    nc.scalar.activation(rz[:, roff:W], rzp[:, roff:W],
                                         Act.Sigmoid)
                    nc.scalar.activation(rz[:, W : W + b], rzp[:, W : W + b],
                                         Act.Sigmoid)
                else:
                    nc.scalar.activation(
                        rz[:, roff : W + b], rzp[:, roff : W + b], Act.Sigmoid
                    )
                rv = rz[:, roff:W]
                zv = rz[:, W : W + b]
                t1 = sp.tile([128, W], F32R, tag="t1")
                if first:
                    # h == 0: (hn + b_hn) * r == b_hn * r
                    nc.vector.tensor_scalar(
                        t1[:, 0:b], rv, b_hn, None, op0=Alu.mult
                    )
                else:
                    nc.vector.scalar_tensor_tensor(
                        t1[:, 0:b], hnp[:, 0:b], b_hn, rv,
                        op0=Alu.add, op1=Alu.mult,
                    )
                inp = pp.tile([128, W], F32, tag="inp")
                nc.tensor.matmul(
                    inp[:, 0:b], wihn[:, 0:128], xt[0:2, 0:b], start=True, stop=False
                )
                nc.tensor.matmul(
                    inp[:, 0:b], ident[:, 0:128], t1[:, 0:b], start=False, stop=True
                )
                # off-critical-path: zc = 1-z (DVE 4x), t3 = z*h (gpsimd)
                zc = sp.tile([128, W], BLD, tag="zc")
                nc.vector.tensor_scalar(
                    zc[:, 0:b], zv, -1.0, 1.0, op0=Alu.mult, op1=Alu.add
                )
                if not first:
                    t3 = sp.tile([128, W], BLD, tag="t3")
                    t3e = nc.gpsimd if CFG["t3_pool"] else nc.vector
                    t3e.tensor_mul(t3[:, 0:b], zv, h[:, 0:b])
                n = sp.tile([128, W], BLD, tag="n")
                nc.scalar.activation(n[:, 0:b], inp[:, 0:b], Act.Tanh, bias=b_in)
                t2 = sp.tile([128, W], BLD, tag="t2")
                nc.vector.tensor_mul(t2[:, 0:b], zc[:, 0:b], n[:, 0:b])
                # leaky(y) with slope 1e-4 ~= relu(y): max abs output error
                # ~1e-5, far inside tolerance; relu is a single cheap
                # tensor_scalar instead of ACT Prelu / DVE STT.
                kpool = CFG["relu_pool_mod"]
                relu_eng = (
                    nc.gpsimd if (kpool and t % kpool == 0) else nc.vector
                )
                if first:
                    relu_eng.tensor_scalar(
                        h[:, 0:b], t2[:, 0:b], 0.0, None, op0=Alu.max
                    )
                else:
                    y = sp.tile([128, W], BLD, tag="y")
                    nc.vector.tensor_add(y[:, 0:b], t2[:, 0:b], t3[:, 0:b])
                    relu_eng.tensor_scalar(
                        h[:, 0:b], y[:, 0:b], 0.0, None, op0=Alu.max
                    )

            def emit_fc(j, fc_pool):
                c0, Wj, Lmax, widths = tiles[j]
                if Lmax == 0:
                    return
                fcp = fc_pool.tile([128, W], F32,
                                   tag="hnp" if CFG["fc_inline"] else "fcp")
                nc.tensor.matmul(
                    fcp[:, 0:Wj], fcw[:, 0:128], hts[j][:, 0:Wj],
                    start=True, stop=True,
                )
                # sigmoid(leaky(v)) ~= max(sigmoid(v), 0.5): for v < 0 the
                # leaky output 1e-4*v maps to sigmoid ~0.5 (max err 7.5e-5).
                le = sp.tile([1, W], F32, tag="le")
                nc.scalar.activation(
                    le[0:1, 0:Wj], fcp[0:1, 0:Wj], Act.Sigmoid,
                    bias=b_fc[0:1, 0:1],
                )
                e = sp.tile([1, W], F32, tag="e")
                nc.vector.tensor_scalar(
                    e[0:1, 0:Wj], le[0:1, 0:Wj], 0.5, None, op0=Alu.max
                )
                nc.sync.dma_start(out_d[0:1, c0 : c0 + Wj], e[0:1, 0:Wj])

            n_tiles = len(tiles)
            GRP = CFG["group"]
            for _rep in range(reps):
                pp_cm = tc.tile_pool(name=f"psum_scan{_rep}", bufs=2,
                                     space="PSUM")
                pp_box["pp"] = pp_cm.__enter__()
                for g0 in range(0, n_tiles, GRP):
                    group = list(range(g0, min(g0 + GRP, n_tiles)))
                    tmax = max(tiles[j][2] for j in group)
                    for t in range(tmax):
                        for j in group:
                            if t < tiles[j][2]:
                                b = tiles[j][3][t]
                                if b > 0:
                                    emit_step(j, t, b)
                                if CFG["fc_inline"] and t == tiles[j][2] - 1:
                                    emit_fc(j, pp_box["pp"])
                    if CFG["fc_inline"]:
                        for j in group:
                            if tiles[j][2] == 0:
                                pass  # zero-length tiles have no fc
                pp_cm.__exit__(None, None, None)

                if not CFG["fc_inline"]:
                    # FC + leaky + sigmoid epilogue at the end
                    pf_cm = tc.tile_pool(name=f"psum_fc{_rep}", bufs=2,
                                         space="PSUM")
                    pf = pf_cm.__enter__()
                    for j in range(n_tiles):
                        emit_fc(j, pf)
                    pf_cm.__exit__(None, None, None)

    return nc


def _run(x, lengths, w_ih, w_hh, b_ih, b_hh, fc_w, fc_b, trace=False):
    from concourse.bass_utils import run_bass_kernel_spmd

    S, T, _ = x.shape
    H = w_hh.shape[1]
    g, M, tiles = _prepare(lengths, T)

    key = (S, T, M, tuple(tiles), float(fc_b[0]))
    if key not in _cache:
        _cache[key] = _build(M, T, tiles, float(fc_b[0]))
    nc = _cache[key]

    # shared weight arrays
    whh_t = np.ascontiguousarray(
        np.concatenate(
            [w_hh[0:H].T, w_hh[H : 2 * H].T, w_hh[2 * H : 3 * H].T], axis=1
        )
    ).astype(np.float32)  # [128, 384] columns r|z|n
    wihb = np.concatenate(
        [
            np.stack([w_ih[0:H, 0], w_ih[0:H, 1], b_ih[0:H] + b_hh[0:H]]),
            np.stack(
                [w_ih[H : 2 * H, 0], w_ih[H : 2 * H, 1],
                 b_ih[H : 2 * H] + b_hh[H : 2 * H]]
            ),
        ],
        axis=1,
    ).astype(np.float32)  # [3, 256]
    wih_n = np.ascontiguousarray(w_ih[2 * H : 3 * H].T).astype(np.float32)  # [2,128]
    ident = np.eye(128, dtype=np.float32)
    fc_wT = np.zeros((128, 128), dtype=np.float32)
    fc_wT[:, 0] = fc_w[0]  # only output row 0 of the FC matmul is read
    biases = np.stack(
        [b_ih[2 * H : 3 * H], b_hh[2 * H : 3 * H],
         np.full(H, fc_b[0], dtype=np.float32)], axis=1
    ).astype(np.float32)  # [128, 3] col0 b_in col1 b_hn col2 fc_b

    if CFG["mm_bf16"]:
        import ml_dtypes

        bf = ml_dtypes.bfloat16
        whh_t = whh_t.astype(bf)
        wihb = wihb.astype(bf)
        wih_n = wih_n.astype(bf)
        ident = ident.astype(bf)
        fc_wT = fc_wT.astype(bf)

    in_maps = []
    core_g = []
    for c in range(NC_N):
        idx = g[c::NC_N]
        core_g.append(idx)
        real = idx >= 0
        xg = np.zeros((M, T, 2), np.float32)
        xg[real] = x[idx[real]]
        xr = np.empty((3, T, M), np.float32)
        xr[0] = xg[:, :, 0].T
        xr[1] = xg[:, :, 1].T
        xr[2] = 1.0
        if CFG["mm_bf16"]:
            import ml_dtypes

            xr = xr.astype(ml_dtypes.bfloat16)
        in_maps.append(
            {
                "x_r": xr,
                "whh_t": whh_t,
                "wihb": wihb,
                "wih_n": wih_n,
                "ident": ident,
                "fc_wT": fc_wT,
                "biases": biases,
            }
        )

    global _last_in_maps
    _last_in_maps = in_maps
    res = run_bass_kernel_spmd(nc, in_maps, list(range(NC_N)), trace=trace)

    out_full = np.zeros(S, dtype=np.float32)
    for c in range(NC_N):
        idx = core_g[c]
        real = idx >= 0
        enc = res.results[c]["out"][0]
        out_full[idx[real]] = enc[real]
    out_full[lengths == 0] = np.float32(0.5)
    return out_full, res


def kernel(**inputs):
    x = np.asarray(inputs["x"], dtype=np.float32)
    lengths = np.asarray(inputs["lengths"], dtype=np.int32)
    w_ih = np.asarray(inputs["w_ih"], dtype=np.float32)
    w_hh = np.asarray(inputs["w_hh"], dtype=np.float32)
    b_ih = np.asarray(inputs["b_ih"], dtype=np.float32)
    b_hh = np.asarray(inputs["b_hh"], dtype=np.float32)
    fc_w = np.asarray(inputs["fc_w"], dtype=np.float32)
    fc_b = np.asarray(inputs["fc_b"], dtype=np.float32)
    out, _ = _run(x, lengths, w_ih, w_hh, b_ih, b_hh, fc_w, fc_b)
    return out

